# revision 29
# baseline (speedup 1.0000x reference)
"""CapsuleLayer dynamic-routing kernel for 8 TRN2 NeuronCores.

Math (per reference):
  priors[c,b,r,o] = sum_i x[b,r,i] * W[c,r,i,o]      b=256, r=1152, i=8, c=10, o=16
  3 routing iterations of softmax(logits over r) -> squash -> logit update.

Sharding: data-parallel over b (8 cores x 32 batch).

Wall time of a dispatch over the axon tunnel is latency/transfer bound
(~85 ms blocking round trip regardless of payload; uploads ~17 ms/MB on
top), so the dispatch path is built for buffer reuse:
  * one warm AOT-compiled shard_map callable (fast_dispatch_compile),
    reused across kernel() calls — run_bass_kernel_spmd would re-trace
    and re-lower (~200 ms) per call;
  * inputs live on device across calls; each call memcmp-verifies the
    caller's arrays against cached host copies and re-packs/re-uploads
    only what changed (x and route_weights independently). The full
    memcmp overlaps the device execution of an optimistically dispatched
    run; a mismatch discards that run and redoes it after the upload.
  * warm calls (unchanged inputs) therefore cost one round trip:
    dispatch + exec + fp16 output fetch, ~88 ms; any failure falls back
    to stock run_bass_kernel_spmd.
Each core receives two packed fp16 dram tensors:
  x2dt [p=(q,j,i), (k,b)]             0.56 MB  this core's batch shard of x
  wsl  [8 k-slice blocks of 128 rows] 2.95 MB  FULL route_weights, replicated
W is replicated (per the sharding hint) in pre-gathered layout — an
earlier revision shipped 1/8 slices and AllGathered on device, but the
per-exec collective made ~30% of warm calls ~40 ms slower (cross-core
straggler sync). On a W change the host ships ONE copy to device 0 and
reshards to replicated (the axon terminal broadcasts device-side, ~1 RT,
vs 23.6 MB of host uploads). The block-diag matmul operand xblk is built
on device from x2dt with 16 strided DMAs into a zeroed tile; the
sum4/expand4 matrices are generated on device with affine_select.

Per-core layout: partition p = 4*b + j where j = r mod 4; r = 4*g + j, g in [0,288).
priors stored in SBUF as fp16 [128, g=288, c=10, o=16].

Per-instruction issue dominates device time in this runtime (~40-100us per
instruction regardless of data size), so everything is shaped to minimize
instruction count: priors are computed by 144 dual-k matmuls (double
block-diag: stationary lhsT = block-diag x [(k2,j,i)=64, (b,jc)=128],
moving rhs = k2-block-diag W [(k2,j,i)=64, (k2c,c,o)=320], PSUM out
[(b,jc)=128, 320] = two k-groups per instruction), drained in 4-pair slabs.
Iteration-0 mean over r via g-chunk DVE reduces of the priors tile
(uniform softmax) that overlap the matmul phase. Cross-partition j-sums /
b-broadcasts via tiny constant matmuls (S = sum4, E = expand4). Softmax
exp in one ACT instruction over all g (no max-subtraction: |logits| <~ 70
fits fp32 range); weighted sums on DVE in 4 g-chunks with strided APs.
Matmul inputs quantized to fp16 (rel err ~2e-4).
"""

import numpy as np

B_FULL, R, I, C, O = 256, 1152, 8, 10, 16
NCORES = 8
B = B_FULL // NCORES          # 32 batch per core
G = R // 4                    # 288 groups of 4 r-values
K72 = R // 16                 # 72 chunks of 16 r (4 groups stacked)
CO = C * O                    # 160
GCHUNK = 72                   # routing g-chunk (big: per-instruction issue
NCHUNK = G // GCHUNK          # 4   overhead ~40us dominates data time)
SLAB = 3                      # priors groups per PSUM bank-slab
KSL = K72 // NCORES           # 9 k-chunks in each core's W upload slice
XC = K72 * B                  # 2304 x2dt cols in blob
WC = KSL * CO                 # 1440 w-slice cols in blob
BLOBC = XC + WC               # 3744

_CACHE = {}


def _build_bass(stage=5, probe_skip=(), wlayout="slice"):
    import concourse.bass as bass
    import concourse.bacc as bacc
    import concourse.mybir as mybir
    from concourse.tile import TileContext
    from contextlib import ExitStack

    f32, f16 = mybir.dt.float32, mybir.dt.float16
    Act = mybir.ActivationFunctionType
    AX, ADD = mybir.AxisListType.X, mybir.AluOpType.add
    GE = mybir.AluOpType.is_ge

    nc = bacc.Bacc("TRN2", target_bir_lowering=False, debug=False,
                   enable_asserts=False, num_devices=NCORES)

    x_d = nc.dram_tensor("x2dt", [128, XC], f16, kind="ExternalInput")
    # W arrives pre-gathered and replicated (sharding_hint: route_weights
    # replicated): row block m holds the k-slice {m + 8*km} in the same
    # layout a device AllGather of per-core slices would produce. Uploading
    # 8x the W bytes only matters when W changes; dropping the per-exec
    # collective removes the cross-core straggler sync from every call.
    if wlayout == "image":
        # exact wbk2 SBUF image (zero-padded k2-block-diag included):
        # loaded with ONE contiguous DMA, no on-device memset/scatter
        w_gath = nc.dram_tensor("wsl", [128, 2 * (K72 // 2) * 2 * CO], f16,
                                kind="ExternalInput")
    else:
        w_gath = nc.dram_tensor("wsl", [NCORES * 128, WC], f16,
                                kind="ExternalInput")
    out_d = nc.dram_tensor("out", [B, CO], f16, kind="ExternalOutput")

    with ExitStack() as ctx:
        tc = ctx.enter_context(TileContext(nc))
        pers = ctx.enter_context(tc.tile_pool(name="pers", bufs=1))

        priors = pers.tile([128, G, C, O], f16)
        logits = pers.tile([128, G, C], f32)
        vexp = pers.tile([128, C, O], f16)
        smat = pers.tile([128, B], f32)
        emat = pers.tile([B, 128], f32)

        # ---- sum4 / expand4 matrices generated on device ----
        # smat[p,b] = 1 iff 0 <= p-4b <= 3 ; emat = smat.T
        nc.vector.memset(smat, 1.0)
        nc.gpsimd.affine_select(smat, smat, pattern=[[-4, B]], compare_op=GE,
                                fill=0.0, base=0, channel_multiplier=1)
        nc.gpsimd.affine_select(smat, smat, pattern=[[4, B]], compare_op=GE,
                                fill=0.0, base=3, channel_multiplier=-1)
        nc.vector.memset(emat, 1.0)
        nc.gpsimd.affine_select(emat, emat, pattern=[[1, 128]], compare_op=GE,
                                fill=0.0, base=0, channel_multiplier=-4)
        nc.gpsimd.affine_select(emat, emat, pattern=[[-1, 128]], compare_op=GE,
                                fill=0.0, base=3, channel_multiplier=4)

        with tc.tile_pool(name="mmin", bufs=1) as mmin:
            # Double block-diag packing: each matmul contracts 64 rows
            # (k2,j,i) and moves 320 cols (k2c,c,o), computing priors for TWO
            # k-values at once — per-instruction issue (~38us) dominates, so
            # halving PE instruction count beats the 2x zero-padding of W.
            # Strip s = q%2 holds rows [64s,64s+64); q2 = q//2 in free dims.
            # xbl2[64s+32k2+8j+i, t, q2, 4b+jc] = x[b, r] if jc==j else 0
            # wbk2[64s+32k2+8j+i, t, q2, k2c, co] = W[c,r,i,o] if k2c==k2
            #   where r = 16(2t+k2) + 4(2q2+s) + j
            TP = K72 // 2  # 36 k-pairs
            xbl2 = mmin.tile([128, 2, TP, 128], f16, name="xbl2")
            wbk2 = mmin.tile([128, 2, TP, 2, CO], f16, name="wbk2")
            if "xmemset" not in probe_skip:
                nc.vector.memset(xbl2, 0.0)
            if wlayout != "image" and "wmemset" not in probe_skip:
                nc.gpsimd.memset(wbk2, 0.0)

            # blob x-section is k2-major (k2, t, b) so each DMA's source
            # collapses to a contiguous run; one DMA per (q2,s,k2,j) keeps
            # every AP within the 3-dim DMA limit
            blob_x = x_d.ap().rearrange("p (k2 t b) -> p k2 t b", k2=2, t=TP)
            xblv = xbl2.rearrange("p q2 t (b jc) -> p q2 t b jc", jc=4)
            blob_xs = blob_x.rearrange("(q2 s2 ji) k2 t b -> ji q2 s2 k2 t b",
                                       q2=2, s2=2)
            di = 0
            for s in range(2):
                for k2 in range(2):
                    for j in range(4):
                        p0 = 64 * s + 32 * k2 + 8 * j
                        for q2 in range(2):
                            if "xdma" in probe_skip:
                                continue
                            eng = nc.sync if di % 2 == 0 else nc.scalar
                            eng.dma_start(
                                out=xblv[p0:p0 + 8, q2, :, :, j],
                                in_=blob_xs[8 * j:8 * j + 8, q2, s, k2, :, :])
                            di += 1

            if wlayout == "image":
                nc.gpsimd.dma_start(
                    out=wbk2.rearrange("p q2 t k2c co -> p (q2 t k2c co)"),
                    in_=w_gath.ap())
            else:
                # Core m holds the strided k-slice {m + 8*km}: k2 = m%2,
                # t = 4*km + m//2 -> dst t-slice m//2::4.
                # w rows = 128m + 32q + (8j+i) = 128m + 64q2 + 32s + rl
                wgv = w_gath.ap().rearrange("(m q2 s rl) kco -> rl q2 s m kco",
                                            m=NCORES, q2=2, s=2)
                for s in range(2):
                    for m in range(NCORES):
                        k2 = m % 2
                        p0 = 64 * s + 32 * k2
                        for q2 in range(2):
                            if "wdma" in probe_skip:
                                continue
                            eng = nc.sync if (m + q2) % 2 == 0 else nc.gpsimd
                            eng.dma_start(
                                out=wbk2[p0:p0 + 32, q2, m // 2::4, k2, :],
                                in_=wgv[:, q2, s, m, :])

            # ---- priors: 288 dual-k matmuls in 36 slabs of 4 pairs ----
            # Slabs keep one row-strip per PSUM bank: concurrent MMs on
            # different row strips must not share a bank (HW crash observed).
            # k-major order so iteration-0 partial sums (reduces over g-chunks
            # below) can start while later k-chunks are still in matmul.
            # pp pool is scoped to this block: priors gets all 8 PSUM banks
            # (sp's routing tiles are only live after the pool closes).
            slabs = []
            if stage >= 2:
                for st in range(TP // 4):
                    for q in range(4):
                        slabs.append((q, st))
            # g = k*4 + q = (2t+k2)*4 + q -> free order (t, k2, q)
            pr_v = priors.rearrange("p (t k2 q) c o -> p q t k2 (c o)", q=4, k2=2)
            with tc.tile_pool(name="pp", bufs=2, space="PSUM") as pp:
                for si, (q, st) in enumerate(slabs):
                    s, q2 = q % 2, q // 2
                    ps = pp.tile([128, 4, 512], f32, tag="slab", name=f"slab{si}")
                    for u in range(4):
                        t = 4 * st + u
                        nc.tensor.matmul(
                            ps[:, u, 0:2 * CO],
                            xbl2[64 * s:64 * s + 64, q2, t, :],
                            wbk2[64 * s:64 * s + 64, q2, t, :, :],
                            start=True, stop=True, tile_position=(64 * s, 0))
                    dst = pr_v[:, q, 4 * st:4 * st + 4, :, :]
                    src = ps[:, :, 0:2 * CO].rearrange("p u (k2 co) -> p u k2 co", co=CO)
                    if si % 2 == 0:
                        nc.scalar.copy(out=dst, in_=src)
                    else:
                        nc.vector.tensor_copy(out=dst, in_=src)

        # rt/sm/sp opened after mmin closes: pools are stack-allocated in
        # open order, and mmin + the big-GCHUNK rt tiles don't fit together
        # (same for the priors pp pool vs sp in PSUM).
        rt = ctx.enter_context(tc.tile_pool(name="rt", bufs=1))
        sm = ctx.enter_context(tc.tile_pool(name="sm", bufs=1))
        sp = ctx.enter_context(tc.tile_pool(name="sp", bufs=1, space="PSUM"))

        # scratch [B, *] f32 slices for squash / normalize temps
        scr = pers.tile([B, 1024], f32)
        s_sb = scr[:, 0:160].rearrange("b (c o) -> b c o", c=C)
        ssq = scr[:, 160:320].rearrange("b (c o) -> b c o", c=C)
        v_sb = scr[:, 320:480].rearrange("b (c o) -> b c o", c=C)
        sq = scr[:, 480:490]
        sqs = scr[:, 490:500]
        den = scr[:, 500:510]
        rden = scr[:, 510:520]
        fsc = scr[:, 520:530]
        rz = scr[:, 540:550]

        sparts = pers.tile([128, NCHUNK, C, O], f32)
        ecf = pers.tile([128, G, C], f32)

        vout = pers.tile([B, C, O], f16)

        def squash_from_s(scale_extra, out=None):
            """v_sb (or `out`) = squash(scale_extra * s_sb)."""
            sc2 = scale_extra * scale_extra
            nc.vector.tensor_mul(ssq, s_sb, s_sb)
            nc.vector.tensor_reduce(sq, ssq, axis=AX, op=ADD)
            nc.scalar.activation(sqs, sq, func=Act.Sqrt, scale=sc2)
            nc.scalar.mul(out=den, in_=sq, mul=sc2)
            nc.scalar.add(out=den, in_=den, add=1.0)
            nc.vector.reciprocal(rden, den)
            nc.vector.tensor_mul(fsc, sqs, rden)
            if scale_extra != 1.0:
                nc.scalar.mul(out=fsc, in_=fsc, mul=scale_extra)
            dst = v_sb if out is None else out
            nc.vector.tensor_mul(dst, s_sb, fsc[:, :, None].broadcast_to([B, C, O]))

        def expand_v():
            """vexp [128, C, O] f16 = replicate v_sb over j."""
            vps = sp.tile([128, CO], f32, tag="vps", bufs=1, name="vps")
            nc.tensor.matmul(vps, emat, v_sb.rearrange("b c o -> b (c o)"),
                             start=True, stop=True)
            nc.scalar.copy(out=vexp.rearrange("p c o -> p (c o)"), in_=vps)

        def delta_acc(first):
            """logits (+)= sum_o priors * vexp. o-reduction as in-place fp16
            halving tree (TT-add at 2x beats tensor_reduce's 1x cap)."""
            for h in range(NCHUNK):
                g0 = h * GCHUNK
                tmp = rt.tile([128, GCHUNK, C, O], f16, tag="dtmp", name=f"dtmp{h}")
                nc.vector.tensor_mul(
                    tmp, priors[:, g0:g0 + GCHUNK],
                    vexp[:, None, :, :].broadcast_to([128, GCHUNK, C, O]))
                for w in (8, 4, 2):
                    nc.vector.tensor_add(tmp[:, :, :, 0:w], tmp[:, :, :, 0:w],
                                         tmp[:, :, :, w:2 * w])
                if first:
                    nc.vector.tensor_add(logits[:, g0:g0 + GCHUNK],
                                         tmp[:, :, :, 0], tmp[:, :, :, 1])
                else:
                    dpart = rt.tile([128, GCHUNK, C], f32, tag="dpart", name=f"dpart{h}")
                    nc.vector.tensor_add(dpart, tmp[:, :, :, 0], tmp[:, :, :, 1])
                    nc.vector.tensor_add(logits[:, g0:g0 + GCHUNK],
                                         logits[:, g0:g0 + GCHUNK], dpart)

        def s_iter(tag):
            """writes s_sb = softmax(logits)-weighted sum of priors (normalized)."""
            nc.scalar.activation(ecf.rearrange("p g c -> p (g c)"),
                                 logits.rearrange("p g c -> p (g c)"),
                                 func=Act.Exp)
            for h in range(NCHUNK):
                g0 = h * GCHUNK
                stmp = rt.tile([128, GCHUNK, C, O], f32, tag="stmp", name=f"stmp{h}")
                nc.vector.tensor_mul(
                    stmp, priors[:, g0:g0 + GCHUNK],
                    ecf[:, g0:g0 + GCHUNK, :, None].broadcast_to([128, GCHUNK, C, O]))
                nc.vector.tensor_reduce(sparts[:, h], stmp.rearrange("p g c o -> p c o g"),
                                        axis=AX, op=ADD)
            sfin = sm.tile([128, C, O], f32, tag="sfin", name="sfin")
            nc.vector.tensor_reduce(sfin, sparts.rearrange("p h c o -> p c o h"),
                                    axis=AX, op=ADD)
            zfin = sm.tile([128, C], f32, tag="zfin", name="zfin")
            nc.vector.tensor_reduce(zfin, ecf.rearrange("p g c -> p c g"),
                                    axis=AX, op=ADD)
            # sj and zj share one PSUM bank (leaves room for pp bufs=3)
            szj = sp.tile([B, 512], f32, tag="sj", bufs=1, name=f"szj{tag}")
            sj_ps = szj[:, 0:CO]
            nc.tensor.matmul(sj_ps, smat, sfin.rearrange("p c o -> p (c o)"),
                             start=True, stop=True)
            zj_ps = szj[:, CO:CO + C]
            nc.tensor.matmul(zj_ps, smat, zfin, start=True, stop=True)
            nc.vector.reciprocal(rz, zj_ps)
            nc.vector.tensor_mul(s_sb, sj_ps.rearrange("b (c o) -> b c o", c=C),
                                 rz[:, :, None].broadcast_to([B, C, O]))

        # ---- iteration 0: s0 = sum_g priors (uniform softmax, z0 = R) ----
        # Partial g-chunk reduces overlap the tail of the matmul phase.
        for h in range(NCHUNK):
            g0 = h * GCHUNK
            nc.vector.tensor_reduce(
                sparts[:, h],
                priors[:, g0:g0 + GCHUNK].rearrange("p g c o -> p c o g"),
                axis=AX, op=ADD)
        sfin0 = sm.tile([128, C, O], f32, tag="sfin", name="sfin0")
        nc.vector.tensor_reduce(sfin0, sparts.rearrange("p h c o -> p c o h"),
                                axis=AX, op=ADD)
        szj0 = sp.tile([B, 512], f32, tag="sj", bufs=1, name="szj0")
        sj0_ps = szj0[:, 0:CO]
        nc.tensor.matmul(sj0_ps, smat, sfin0.rearrange("p c o -> p (c o)"),
                         start=True, stop=True)
        nc.vector.tensor_copy(out=s_sb, in_=sj0_ps.rearrange("b (c o) -> b c o", c=C))
        squash_from_s(1.0 / R)
        if stage >= 3:
            expand_v()
            delta_acc(first=True)
        if stage >= 4:
            # ---- iteration 1 ----
            s_iter("1")
            squash_from_s(1.0)
        if stage >= 5:
            expand_v()
            delta_acc(first=False)
            # ---- iteration 2 ----
            s_iter("2")
            squash_from_s(1.0, out=vout)
        else:
            nc.vector.tensor_copy(out=vout, in_=v_sb)
        nc.sync.dma_start(out=out_d.ap(), in_=vout.rearrange("b c o -> b (c o)"))

    nc.finalize()
    return nc


def _pack_x_core(x, m):
    # x2dt[(q,j,i), (k2, t, b)] = x[32m+b, 16k+4q+j, i], k = 2t+k2
    xt = (x[B * m:B * (m + 1)].reshape(B, K72, 128)
          .transpose(2, 1, 0).astype(np.float16))
    return (xt.reshape(128, K72 // 2, 2, B)
            .transpose(0, 2, 1, 3).reshape(128, XC))


def _pack_w_core(W, m):
    # wsl block m: [(q,j,i), k_local, (c,o)] = W[c, 16k+4q+j, i, o] for the
    # strided k-slice k = m + 8*k_local (k2-major: k2 = m%2 constant)
    wblk = (W.reshape(C, K72, 128, O)[:, m::NCORES]
            .transpose(2, 1, 0, 3).astype(np.float16))
    return np.ascontiguousarray(wblk).reshape(128, WC)


def _pack_w_full(W):
    # pre-gathered replicated W: row block m = _pack_w_core(W, m)
    return np.concatenate([_pack_w_core(W, m) for m in range(NCORES)], axis=0)


def _pack_w_img(W):
    # exact wbk2 SBUF image [128, (q2 t k2c co)] incl. k2-block-diag zeros:
    # wbk2[64s+32k2+rl, q2, t, k2c, co] = slice block if k2c==k2 else 0
    TP = K72 // 2
    img = np.zeros((128, 2, TP, 2, CO), np.float16)
    wv = _pack_w_full(W).reshape(NCORES, 2, 2, 32, KSL, CO)  # (m,q2,s,rl,kl,co)
    for m in range(NCORES):
        k2 = m % 2
        for s in range(2):
            p0 = 64 * s + 32 * k2
            for q2 in range(2):
                img[p0:p0 + 32, q2, m // 2::4, k2, :] = wv[m, q2, s]
    return img.reshape(128, 2 * TP * 2 * CO)


def _build_fast_path(nc):
    """Warm AOT dispatch for the Bass NEFF — the same execution path
    run_bass_kernel_spmd takes under axon (bass2jax -> _bass_exec_p ->
    PJRT shard_map over 8 cores), inlined so the compiled callable and the
    device-resident input can be reused across kernel() calls.
    run_bass_kernel_spmd rebuilds jax.jit(shard_map(...)) per call
    (~200ms retrace) and re-ships every operand from host numpy (~215ms
    for the 7.7MB blob); with unchanged inputs both are pure overhead.
    The donated zero output operand is dropped: this kernel writes every
    element of `out`, and with no input/output alias declared the NKI
    lowering allocates a fresh shared_hbm buffer for it. The callable is
    built via fast_dispatch_compile (BassEffect suppressed -> C++
    fast-path dispatch)."""
    import jax
    import jax.numpy as jnp
    from jax.sharding import Mesh, PartitionSpec, NamedSharding
    from jax.experimental.shard_map import shard_map
    from concourse.bass2jax import (_bass_exec_p, install_neuronx_cc_hook,
                                    partition_id_tensor, fast_dispatch_compile)
    import concourse.mybir as mybir

    install_neuronx_cc_hook()
    partition_name = nc.partition_id_tensor.name if nc.partition_id_tensor else None
    in_names, out_names, out_avals = [], [], []
    for alloc in nc.m.functions[0].allocations:
        if not isinstance(alloc, mybir.MemoryLocationSet):
            continue
        name = alloc.memorylocations[0].name
        if alloc.kind == "ExternalInput":
            if name != partition_name:
                in_names.append(name)
        elif alloc.kind == "ExternalOutput":
            out_names.append(name)
            out_avals.append(jax.core.ShapedArray(tuple(alloc.tensor_shape),
                                                  mybir.dt.np(alloc.dtype)))
    assert sorted(in_names) == ["wsl", "x2dt"] and out_names == ["out"]
    bind_names = ("x2dt", "wsl") + ((partition_name,) if partition_name else ())

    def _body(x_shard, w_shard):
        operands = [x_shard, w_shard]
        if partition_name is not None:
            operands.append(partition_id_tensor())
        return tuple(_bass_exec_p.bind(
            *operands, out_avals=tuple(out_avals), in_names=bind_names,
            out_names=tuple(out_names), lowering_input_output_aliases=(),
            sim_require_finite=True, sim_require_nnan=True, nc=nc))

    devices = jax.devices()[:NCORES]
    assert len(devices) >= NCORES
    mesh = Mesh(np.asarray(devices), ("core",))
    sh = NamedSharding(mesh, PartitionSpec("core"))
    wrapped = shard_map(_body, mesh=mesh,
                        in_specs=(PartitionSpec("core"),) * 2,
                        out_specs=(PartitionSpec("core"),), check_rep=False)
    wsl_shape = None
    for alloc in nc.m.functions[0].allocations:
        if (isinstance(alloc, mybir.MemoryLocationSet)
                and alloc.kind == "ExternalInput"
                and alloc.memorylocations[0].name == "wsl"):
            wsl_shape = tuple(alloc.tensor_shape)
    x_sds = jax.ShapeDtypeStruct((NCORES * 128, XC), jnp.float16, sharding=sh)
    w_sds = jax.ShapeDtypeStruct((NCORES * wsl_shape[0], wsl_shape[1]),
                                 jnp.float16, sharding=sh)
    compiled = fast_dispatch_compile(
        lambda: jax.jit(wrapped).lower(x_sds, w_sds).compile())

    return {"jax": jax, "compiled": compiled, "sh": sh,
            "devices": devices, "mesh": mesh, "wsl_shape": wsl_shape}


def _run_fallback(nc, x, route_weights):
    from concourse.bass_utils import run_bass_kernel_spmd
    wfull = _pack_w_full(route_weights)
    in_maps = [{"x2dt": _pack_x_core(x, m), "wsl": wfull}
               for m in range(NCORES)]
    try:
        res = run_bass_kernel_spmd(nc, in_maps, core_ids=list(range(NCORES)))
    except Exception:
        # rare NRT_EXEC_UNIT_UNRECOVERABLE: the next dispatch triggers a
        # device reset (~3 min) and then succeeds
        res = run_bass_kernel_spmd(nc, in_maps, core_ids=list(range(NCORES)))
    _CACHE["last_results"] = res
    return np.stack([res.results[m]["out"] for m in range(NCORES)])


def kernel(x, route_weights):
    import time as _time

    if "nc" not in _CACHE:
        _CACHE["nc"] = _build_bass()
    nc = _CACHE["nc"]

    x = np.asarray(x)
    route_weights = np.asarray(route_weights)

    if "fp" not in _CACHE:
        if _CACHE.get("fp_fail", 0) >= 2:
            _CACHE["fp"] = None          # repeated failures: stay on stock path
        else:
            try:
                _CACHE["fp"] = _build_fast_path(nc)
            except Exception:
                _CACHE["fp"] = None
    fp = _CACHE["fp"]

    if fp is None:
        _t0 = _time.time()
        out = _run_fallback(nc, x, route_weights)
        _CACHE["last_run_wall_s"] = _time.time() - _t0
        return out.astype(np.float32).reshape(B_FULL, C, O)

    # Device-resident input cache, per tensor: x and route_weights are
    # cached/re-uploaded independently (weights usually stay put while
    # activations change). A strided sample gates the optimistic dispatch
    # (~0.1ms); the FULL memcmp runs while the device executes, and on a
    # mismatch the speculative result is discarded and the changed tensor
    # is repacked + re-uploaded. The Bass program never writes its input
    # dram tensors, so cached buffers survive dispatches. Uploads are
    # chunked per core so numpy packing overlaps the wire transfer.
    def _cheap_same(arr, key, sample):
        ref = _CACHE.get(key)
        return (ref is not None and arr.shape == ref.shape
                and np.array_equal(sample(arr), sample(ref)))

    def _full_same(arr, key):
        return np.array_equal(arr, _CACHE[key])

    from jax.sharding import SingleDeviceSharding

    def _upload_x():
        shards = []
        for m in range(NCORES):
            part = _pack_x_core(x, m)                # ~3ms numpy
            shards.append(fp["jax"].device_put(      # async: wire starts now
                part, SingleDeviceSharding(fp["devices"][m])))
        _CACHE["d_x"] = fp["jax"].make_array_from_single_device_arrays(
            (NCORES * 128, XC), fp["sh"], shards)
        _CACHE["x_ref"] = x.copy()

    def _upload_w():
        # ship one 2.95MB copy to device 0, then reshard to replicated —
        # the axon terminal does the 8-way broadcast device-side (~1 RT)
        # instead of 8 host uploads (23.6MB, ~500ms)
        from jax.sharding import NamedSharding, PartitionSpec
        wfull = _pack_w_full(route_weights)
        d0 = fp["jax"].device_put(wfull, SingleDeviceSharding(fp["devices"][0]))
        repl = fp["jax"].device_put(
            d0, NamedSharding(fp["mesh"], PartitionSpec()))
        by_dev = {s.device: s.data for s in repl.addressable_shards}
        r0, r1 = fp["wsl_shape"]
        _CACHE["d_w"] = fp["jax"].make_array_from_single_device_arrays(
            (NCORES * r0, r1), fp["sh"],
            [by_dev[d] for d in fp["devices"]])
        _CACHE["w_ref"] = route_weights.copy()

    def _dispatch():
        return fp["compiled"](_CACHE["d_x"], _CACHE["d_w"])

    try:
        _t0 = _time.time()
        x_cheap = _cheap_same(x, "x_ref", lambda a: a[::37])
        w_cheap = _cheap_same(route_weights, "w_ref", lambda a: a[::3, ::101])
        if not x_cheap:
            _upload_x()
        if not w_cheap:
            _upload_w()
        out_dev = _dispatch()                             # async dispatch
        redo = False
        if x_cheap and not _full_same(x, "x_ref"):        # overlapped verify
            _upload_x()
            redo = True
        if w_cheap and not _full_same(route_weights, "w_ref"):
            _upload_w()
            redo = True
        if redo:
            out_dev = _dispatch()
        out = np.asarray(out_dev[0])                      # blocks: exec+fetch
        _CACHE["last_run_wall_s"] = _time.time() - _t0
        return out.astype(np.float32).reshape(B_FULL, C, O)
    except Exception:
        # any fast-path hiccup: invalidate, take the stock path this call,
        # and let the next call rebuild the fast path (max 2 rebuilds)
        _CACHE.pop("fp", None)
        _CACHE["fp_fail"] = _CACHE.get("fp_fail", 0) + 1
        for k in ("d_x", "d_w", "x_ref", "w_ref"):
            _CACHE.pop(k, None)
        _t0 = _time.time()
        out = _run_fallback(nc, x, route_weights)
        _CACHE["last_run_wall_s"] = _time.time() - _t0
        return out.astype(np.float32).reshape(B_FULL, C, O)

